# revision 1
# baseline (speedup 1.0000x reference)
import jax
import jax.numpy as jnp
import numpy as np
from functools import partial

# Hardcoded problem shapes (nn_Block sparse_attention):
#   x: (8, 8192, 256); HEADS=8, LOCAL_HEADS=1, WINDOW=64, HD=32; MLP_HIDDEN=1024
B, N, DIM = 8, 8192, 256
HEADS, LOCAL_HEADS, WINDOW = 8, 1, 64
HD = DIM // HEADS  # 32
MLP_HIDDEN = 1024
EPS = 1e-5


def _layer_norm(x, g, b):
    mu = jnp.mean(x, axis=-1, keepdims=True)
    var = jnp.var(x, axis=-1, keepdims=True)
    return (x - mu) * jax.lax.rsqrt(var + EPS) * g + b


def _look_around(t, pad_value=0.0):
    nw = t.shape[1]
    pads = [(0, 0), (1, 1)] + [(0, 0)] * (t.ndim - 2)
    p = jnp.pad(t, pads, constant_values=pad_value)
    return jnp.concatenate([p[:, i:i + nw] for i in range(3)], axis=2)


def _local_attn(q, k, v):
    b, n, d = q.shape
    nw = n // WINDOW
    scale = d ** -0.5
    bq = q.reshape(b, nw, WINDOW, d) * scale
    bk = _look_around(k.reshape(b, nw, WINDOW, d))
    bv = _look_around(v.reshape(b, nw, WINDOW, d))
    ticker = jnp.arange(n).reshape(1, nw, WINDOW)
    bq_k = _look_around(ticker, pad_value=-1)
    pad_mask = (bq_k == -1)[:, :, None, :]
    dots = jnp.einsum('bnie,bnje->bnij', bq, bk)
    dots = jnp.where(pad_mask, -1e9, dots)
    attn = jax.nn.softmax(dots, axis=-1)
    out = jnp.einsum('bnij,bnje->bnie', attn, bv)
    return out.reshape(b, n, d)


def _linear_attn(q, k, v):
    d = q.shape[-1]
    qs = jax.nn.softmax(q, axis=-1) * d ** -0.5
    ks = jax.nn.softmax(k, axis=-2)
    ctx = jnp.einsum('bhnd,bhne->bhde', ks, v)
    return jnp.einsum('bhnd,bhde->bhne', qs, ctx)


def _block(x, norm1_g, norm1_b, Wq, Wkv, Wproj, bproj, norm2_g, norm2_b,
           W1, b1, W2, b2):
    # x here is the per-device shard: (B/8, N, DIM) = (1, 8192, 256)
    b, n, c = x.shape
    h = _layer_norm(x, norm1_g, norm1_b)
    q = h @ Wq.T
    kv = h @ Wkv.T
    split_heads = lambda t: t.reshape(b, n, HEADS, HD).transpose(0, 2, 1, 3)
    q, k, v = split_heads(q), split_heads(kv), split_heads(kv)
    lq, gq = q[:, :LOCAL_HEADS], q[:, LOCAL_HEADS:]
    lk, gk = k[:, :LOCAL_HEADS], k[:, LOCAL_HEADS:]
    lv, gv = v[:, :LOCAL_HEADS], v[:, LOCAL_HEADS:]
    lout = _local_attn(lq.reshape(b * LOCAL_HEADS, n, HD),
                       lk.reshape(b * LOCAL_HEADS, n, HD),
                       lv.reshape(b * LOCAL_HEADS, n, HD)).reshape(b, LOCAL_HEADS, n, HD)
    gout = _linear_attn(gq, gk, gv)
    attn = jnp.concatenate([lout, gout], axis=1)
    y = attn.transpose(0, 2, 1, 3).reshape(b, n, c) @ Wproj.T + bproj
    x = x + y
    h2 = _layer_norm(x, norm2_g, norm2_b)
    m = jax.nn.gelu(h2 @ W1.T + b1, approximate=False) @ W2.T + b2
    return x + m


_pmapped = None


def _get_pmapped():
    global _pmapped
    if _pmapped is None:
        # Data-parallel over batch: 8 batch elements -> 8 cores, weights
        # replicated (in_axes=None broadcasts).
        _pmapped = jax.pmap(
            _block,
            in_axes=(0,) + (None,) * 12,
            devices=jax.devices()[:8],
        )
    return _pmapped


def kernel(x, norm1_g, norm1_b, Wq, Wkv, Wproj, bproj, norm2_g, norm2_b,
           W1, b1, W2, b2):
    x = np.asarray(x, dtype=np.float32)
    fn = _get_pmapped()
    # shard batch: (8, N, DIM) -> (8 devices, 1, N, DIM)
    xs = x.reshape(B, 1, N, DIM)
    out = fn(xs,
             jnp.asarray(norm1_g), jnp.asarray(norm1_b),
             jnp.asarray(Wq), jnp.asarray(Wkv), jnp.asarray(Wproj),
             jnp.asarray(bproj), jnp.asarray(norm2_g), jnp.asarray(norm2_b),
             jnp.asarray(W1), jnp.asarray(b1), jnp.asarray(W2), jnp.asarray(b2))
    out = np.asarray(out).reshape(B, N, DIM).astype(np.float32)
    return out


if __name__ == "__main__":
    key = jax.random.key(0)
    ks = jax.random.split(key, 8)
    s = 0.02
    inputs = {
        'x': np.asarray(jax.random.normal(ks[0], (B, N, DIM), jnp.float32)),
        'norm1_g': np.ones((DIM,), np.float32),
        'norm1_b': np.zeros((DIM,), np.float32),
        'Wq': np.asarray(jax.random.normal(ks[1], (DIM, DIM), jnp.float32)) * s,
        'Wkv': np.asarray(jax.random.normal(ks[2], (DIM, DIM), jnp.float32)) * s,
        'Wproj': np.asarray(jax.random.normal(ks[3], (DIM, DIM), jnp.float32)) * s,
        'bproj': np.zeros((DIM,), np.float32),
        'norm2_g': np.ones((DIM,), np.float32),
        'norm2_b': np.zeros((DIM,), np.float32),
        'W1': np.asarray(jax.random.normal(ks[4], (MLP_HIDDEN, DIM), jnp.float32)) * s,
        'b1': np.zeros((MLP_HIDDEN,), np.float32),
        'W2': np.asarray(jax.random.normal(ks[5], (DIM, MLP_HIDDEN), jnp.float32)) * s,
        'b2': np.zeros((DIM,), np.float32),
    }
    out = kernel(**inputs)
    print("out shape:", out.shape, "dtype:", out.dtype)
